# revision 1
# baseline (speedup 1.0000x reference)
"""Attention-pooling kernel for Trainium2 (raw Bass), SPMD over 8 NeuronCores.

Computation (per sample b):
    score[t] = tanh(sum_d X[b,t,d] * w[d] + bias[t])
    out[b,d] = sum_t softmax_t(score)[t] * X[b,t,d]

Sharding: data-parallel over batch (32 samples -> 4 per core); w/bias replicated.

v2 design (fp16 upload, score pass split across DVE and ACT):
  - X is uploaded in fp16 (host cast inside kernel()), halving HBM traffic to
    16 MiB/core (measured end-to-end rel err ~3e-4 vs 2e-2 tolerance).
    t-rows are mapped t = p*16 + c so each partition's per-sample DMA slice is
    contiguous (16 KB descriptors instead of 4 KB; softmax/pooling are
    t-permutation invariant, bias is loaded with the same permutation).
  - score: HW-probed costs (drift-cancelled interleaved A/B): fused
    scalar_tensor_tensor is 1x-only (~0.94 us/chunk, any dtype), plain
    tensor_tensor hits the 2x 16-bit mode (~0.48 us/chunk), ACT
    Identity-with-accum reduces a chunk in ~1.0 us. So per sample the 16
    t-chunks split: 7 chunks fused stt on DVE; 9 chunks TT-multiply on DVE
    into a 4-slot fp16 product ring, reduced on ACT (Identity, accum_out)
    with bias folded in via bias/D as the ACT per-partition bias. This
    balances DVE ~43 us and ACT ~42 us per pass vs the old DVE-only 64 us.
  - An explicit self-semaphore wait is required between an stt and the next
    reader of its accum_out: the accumulator result lands after instruction
    retire, and skipping the wait produced ~0.17 rel err (race confirmed on
    HW both for f32 and 16-bit stt).
  - softmax without max-subtraction (tanh output is in [-1,1], exp is safe):
    ACT tanh then exp per 8-chunk group with fused row-sum accum; PE
    accumulates the cross-partition total via matmuls against a ones column;
    DVE reciprocal.
  - pooling: PE matmuls esc[:,c] (fp16) x X_chunk (fp16) accumulated in PSUM
    [1,1024]; the 1/sum(exp) normalization is folded into the PSUM->SBUF copy
    as the ACT per-partition scale. The 4 output rows are staged in one SBUF
    row and written with a single DMA per pass.
Everything is double-buffered by sample parity; engines sync with explicit
semaphores. Steady state is DVE/ACT-bound at ~43 us per pass (DMA 31, PE 35);
~1.3x faster than the f32 DVE-bound v1 (64204 ns harness baseline).
"""

import numpy as np

import concourse.bass as bass
import concourse.mybir as mybir
from concourse.bass_utils import run_bass_kernel_spmd

B, T, D = 32, 2048, 1024
N_CORES = 8
BPC = B // N_CORES  # samples per core
P = 128
NCHUNK = T // P  # 16
NGROUP = 2
CPG = NCHUNK // NGROUP  # 8

f32 = mybir.dt.float32
fp16 = mybir.dt.float16
Tanh = mybir.ActivationFunctionType.Tanh
Exp = mybir.ActivationFunctionType.Exp
Copy = mybir.ActivationFunctionType.Copy
Identity = mybir.ActivationFunctionType.Identity
Alu = mybir.AluOpType

Q0, Q1 = 4, 3  # fused-stt chunks in group 0 / group 1 (rest: TT+ACT-reduce)
NSLOT = 6  # fp16 product ring depth (DVE TT -> ACT reduce)


def _build_nc(reps: int = 1, q0: int = Q0, q1: int = Q1, nslot: int = NSLOT, split_out: bool = True, defer: bool = False) -> bass.Bass:
    QS = [list(range(0, q0)), list(range(CPG, CPG + q1))]
    MS = [list(range(q0, CPG)), list(range(CPG + q1, NCHUNK))]

    nc = bass.Bass("TRN2", target_bir_lowering=False, debug=False)
    x = nc.dram_tensor("x", [BPC, T, D], fp16, kind="ExternalInput").ap()
    w = nc.dram_tensor("w", [D, 1], fp16, kind="ExternalInput")
    bias = nc.dram_tensor("bias", [T, 1], f32, kind="ExternalInput")
    bias_s = nc.dram_tensor("bias_s", [T, 1], f32, kind="ExternalInput")  # bias/D
    out = nc.dram_tensor("out", [BPC, D], f32, kind="ExternalOutput").ap()

    NS = BPC * reps

    from contextlib import ExitStack

    with ExitStack() as es:
        ec = es.enter_context
        xt = [ec(nc.sbuf_tensor(f"xt{i}", [P, NCHUNK, D], fp16)) for i in range(2)]
        wt = ec(nc.sbuf_tensor("wt", [P, D], fp16))
        bias_t = ec(nc.sbuf_tensor("bias_t", [P, NCHUNK], f32))
        bias_st = ec(nc.sbuf_tensor("bias_st", [P, NCHUNK], f32))
        ones_col = ec(nc.sbuf_tensor("ones_col", [P, 1], f32))
        prod = ec(nc.sbuf_tensor("prod", [P, nslot, D], fp16))
        scrap = ec(nc.sbuf_tensor("scrap", [P, 8], fp16))
        score = [ec(nc.sbuf_tensor(f"score{i}", [P, NCHUNK], f32)) for i in range(2)]
        esc = [ec(nc.sbuf_tensor(f"esc{i}", [P, NCHUNK], fp16)) for i in range(2)]
        sumexp = [ec(nc.sbuf_tensor(f"sumexp{i}", [P, NGROUP], f32)) for i in range(2)]
        recip = [ec(nc.sbuf_tensor(f"recip{i}", [1, 1], f32)) for i in range(2)]
        orow = [ec(nc.sbuf_tensor(f"orow{i}", [1, BPC * D], f32)) for i in range(2)]
        pa = [ec(nc.psum_tensor(f"pool_a{i}", [1, 512], f32)) for i in range(2)]
        pb = [ec(nc.psum_tensor(f"pool_b{i}", [1, 512], f32)) for i in range(2)]
        tot = [ec(nc.psum_tensor(f"tot{i}", [1, 1], f32)) for i in range(2)]
        cset = ec(nc.semaphore("cset"))
        dma_in_s = [
            [ec(nc.semaphore(f"dma_in{p}{g}")) for g in range(NGROUP)]
            for p in range(2)
        ]
        prod_sem = ec(nc.semaphore("prod_sem"))  # DVE TT-mult done (per M chunk)
        red_sem = ec(nc.semaphore("red_sem"))  # ACT reduce done (per M chunk)
        sttb_sem = ec(nc.semaphore("sttb_sem"))  # stt accum landed (per Q chunk)
        qb_sem = ec(nc.semaphore("qb_sem"))  # DVE Q-cols biased (per group)
        act_sem = ec(nc.semaphore("act_sem"))  # ACT exp done (per group)
        pe_tot = ec(nc.semaphore("pe_tot"))
        recip_sem = ec(nc.semaphore("recip_sem"))
        pe_pool = ec(nc.semaphore("pe_pool"))
        act_out = ec(nc.semaphore("act_out"))
        ones_sem = ec(nc.semaphore("ones_sem"))
        dma_out_s = [ec(nc.semaphore(f"dma_out{i}")) for i in range(2)]
        block = ec(nc.Block())

        @block.gpsimd
        def _(gpsimd):
            gpsimd.dma_start(
                wt[:], bass.AP(tensor=w, offset=0, ap=[[0, P], [1, D]])
            ).then_inc(cset, 16)
            bap = [[NCHUNK, P], [1, NCHUNK]]  # bias_t[p, c] = bias[p*16 + c]
            gpsimd.dma_start(
                bias_t[:], bass.AP(tensor=bias, offset=0, ap=bap)
            ).then_inc(cset, 16)
            gpsimd.dma_start(
                bias_st[:], bass.AP(tensor=bias_s, offset=0, ap=bap)
            ).then_inc(cset, 16)
            gpsimd.memset(ones_col[:], 1.0).then_inc(ones_sem, 1)
            for v in range(NS):
                s, p_ = v % BPC, v % 2
                xs = x[s].rearrange("(p c) d -> p c d", c=NCHUNK)
                if v >= 2:
                    gpsimd.wait_ge(pe_pool, v - 1)  # xt[p_] free (pooling v-2 done)
                for g in range(NGROUP):
                    if v == 0 and g == 0:
                        h = CPG // 2
                        gpsimd.dma_start(
                            out=xt[p_][:, 0:h, :], in_=xs[:, 0:h, :]
                        ).then_inc(dma_in_s[p_][g], 16)
                        gpsimd.dma_start(
                            out=xt[p_][:, h:CPG, :], in_=xs[:, h:CPG, :]
                        ).then_inc(dma_in_s[p_][g], 16)
                    else:
                        gpsimd.dma_start(
                            out=xt[p_][:, g * CPG : (g + 1) * CPG, :],
                            in_=xs[:, g * CPG : (g + 1) * CPG, :],
                        ).then_inc(dma_in_s[p_][g], 16)

        @block.sync
        def _(sync):
            H = BPC // 2
            for r in range(reps):
                rp = r % 2
                if split_out:
                    sync.wait_ge(act_out, BPC * r + H)
                    sync.dma_start(
                        out=out[0:H, :], in_=orow[rp][:, 0 : H * D]
                    ).then_inc(dma_out_s[rp], 16)
                    sync.wait_ge(act_out, BPC * (r + 1))
                    sync.dma_start(
                        out=out[H:BPC, :], in_=orow[rp][:, H * D : BPC * D]
                    ).then_inc(dma_out_s[rp], 16)
                else:
                    sync.wait_ge(act_out, BPC * (r + 1))
                    sync.dma_start(out=out[:, :], in_=orow[rp][:]).then_inc(
                        dma_out_s[rp], 16
                    )
            m = 32 if split_out else 16
            sync.wait_ge(dma_out_s[0], m * ((reps + 1) // 2))
            if reps > 1:
                sync.wait_ge(dma_out_s[1], m * (reps // 2))

        @block.vector
        def _(vector):
            vector.wait_ge(cset, 48)
            k_tt = 0
            k_stt = 0
            for v in range(NS):
                s, p_ = v % BPC, v % 2
                for g in range(NGROUP):
                    ex0 = 16 if (p_ == 0 and g == 0) else 0
                    if v == 0 and g == 0:
                        vector.wait_ge(dma_in_s[p_][g], 16)  # chunks 0-3 landed
                        for c in QS[g]:
                            nc.vector.scalar_tensor_tensor(
                                out=scrap[:, c % 8 : c % 8 + 1].broadcast_to((P, D)),
                                in0=xt[p_][:, c, :],
                                scalar=0.0,
                                in1=wt[:],
                                op0=Alu.bypass,
                                op1=Alu.mult,
                                accum_out=score[p_][:, c : c + 1],
                            ).then_inc(sttb_sem, 1)
                            k_stt += 1
                        vector.wait_ge(dma_in_s[p_][g], 32)  # chunks 4-7 landed
                        for c in MS[g]:
                            nc.vector.tensor_tensor(
                                out=prod[:, k_tt % nslot, :],
                                in0=xt[p_][:, c, :],
                                in1=wt[:],
                                op=Alu.mult,
                            ).then_inc(prod_sem, 1)
                            k_tt += 1
                        vector.wait_ge(sttb_sem, k_stt)
                        qs = slice(QS[g][0], QS[g][-1] + 1)
                        nc.vector.tensor_tensor(
                            out=score[p_][:, qs],
                            in0=score[p_][:, qs],
                            in1=bias_t[:, qs],
                            op=Alu.add,
                        ).then_inc(qb_sem, 1)
                        continue
                    vector.wait_ge(dma_in_s[p_][g], 16 * (v // 2 + 1) + ex0)
                    if g == 0 and v >= 2:
                        # score/esc of v-2 fully consumed (exp emitted act_sem)
                        vector.wait_ge(act_sem, NGROUP * (v - 1))
                    # TT-multiplies first so ACT's reduces start early
                    for c in MS[g]:
                        if k_tt >= nslot:
                            vector.wait_ge(red_sem, k_tt - nslot + 1)
                        nc.vector.tensor_tensor(
                            out=prod[:, k_tt % nslot, :],
                            in0=xt[p_][:, c, :],
                            in1=wt[:],
                            op=Alu.mult,
                        ).then_inc(prod_sem, 1)
                        k_tt += 1
                    for c in QS[g]:
                        nc.vector.scalar_tensor_tensor(
                            out=scrap[:, c % 8 : c % 8 + 1].broadcast_to((P, D)),
                            in0=xt[p_][:, c, :],
                            scalar=0.0,
                            in1=wt[:],
                            op0=Alu.bypass,
                            op1=Alu.mult,
                            accum_out=score[p_][:, c : c + 1],
                        ).then_inc(sttb_sem, 1)
                        k_stt += 1
                    # accum_out lands after retire; must see our own writes
                    vector.wait_ge(sttb_sem, k_stt)
                    qs = slice(QS[g][0], QS[g][-1] + 1)
                    nc.vector.tensor_tensor(
                        out=score[p_][:, qs],
                        in0=score[p_][:, qs],
                        in1=bias_t[:, qs],
                        op=Alu.add,
                    ).then_inc(qb_sem, 1)
                if v >= 1:
                    pv, pp = v - 1, (v - 1) % 2
                    vector.wait_ge(pe_tot, pv + 1)
                    if pv >= 2:
                        vector.wait_ge(act_out, pv - 1)  # recip[pp] free
                    nc.vector.reciprocal(out=recip[pp][:], in_=tot[pp][:]).then_inc(
                        recip_sem, 1
                    )
            pv, pp = NS - 1, (NS - 1) % 2
            vector.wait_ge(pe_tot, pv + 1)
            if pv >= 2:
                vector.wait_ge(act_out, pv - 1)
            nc.vector.reciprocal(out=recip[pp][:], in_=tot[pp][:]).then_inc(
                recip_sem, 1
            )

        def _emit_copies(scalar, v):
            s, p_ = v % BPC, v % 2
            r, rp = v // BPC, (v // BPC) % 2
            scalar.wait_ge(pe_pool, v + 1)
            scalar.wait_ge(recip_sem, v + 1)
            if s == 0 and r >= 2:
                scalar.wait_ge(dma_out_s[rp], (32 if split_out else 16) * (r // 2))
            o0 = s * D
            nc.scalar.activation(
                out=orow[rp][:, o0 : o0 + 512], in_=pa[p_][:], func=Copy,
                scale=recip[p_][:],
            )
            nc.scalar.activation(
                out=orow[rp][:, o0 + 512 : o0 + 1024], in_=pb[p_][:], func=Copy,
                scale=recip[p_][:],
            ).then_inc(act_out, 1)

        @block.scalar
        def _(scalar):
            k_red = 0

            def _red(v, g, p_):
                nonlocal k_red
                for c in MS[g]:
                    scalar.wait_ge(prod_sem, k_red + 1)
                    nc.scalar.activation(
                        out=prod[:, k_red % nslot, :],
                        in_=prod[:, k_red % nslot, :],
                        func=Identity,
                        bias=bias_st[:, c : c + 1],
                        accum_out=score[p_][:, c : c + 1],
                    ).then_inc(red_sem, 1)
                    k_red += 1

            def _softmax(v, g, p_):
                gs = slice(g * CPG, (g + 1) * CPG)
                scalar.wait_ge(qb_sem, NGROUP * v + g + 1)
                nc.scalar.activation(
                    out=score[p_][:, gs], in_=score[p_][:, gs], func=Tanh
                )
                nc.scalar.activation(
                    out=esc[p_][:, gs],
                    in_=score[p_][:, gs],
                    func=Exp,
                    accum_out=sumexp[p_][:, g : g + 1],
                ).then_inc(act_sem, 1)

            for v in range(NS):
                s, p_ = v % BPC, v % 2
                if v >= 2:
                    scalar.wait_ge(pe_pool, v - 1)  # esc/sumexp[p_] free
                if defer:
                    _red(v, 0, p_)
                    _red(v, 1, p_)
                    _softmax(v, 0, p_)
                    _softmax(v, 1, p_)
                else:
                    _red(v, 0, p_)
                    _softmax(v, 0, p_)
                    _red(v, 1, p_)
                    _softmax(v, 1, p_)
                if v >= 1:
                    _emit_copies(scalar, v - 1)
            _emit_copies(scalar, NS - 1)

        @block.tensor
        def _(tensor):
            tensor.wait_ge(ones_sem, 1)
            for v in range(NS):
                s, p_ = v % BPC, v % 2
                for g in range(NGROUP):
                    tensor.wait_ge(act_sem, NGROUP * v + g + 1)
                    if g == 0 and v >= 2:
                        tensor.wait_ge(recip_sem, v - 1)  # tot[p_] free
                        tensor.wait_ge(act_out, v - 1)  # pa/pb[p_] free
                    mm_t = nc.tensor.matmul(
                        tot[p_][:],
                        sumexp[p_][:, g : g + 1],
                        ones_col[:],
                        start=(g == 0),
                        stop=(g == NGROUP - 1),
                    )
                    if g == NGROUP - 1:
                        mm_t.then_inc(pe_tot, 1)
                    for c in range(g * CPG, (g + 1) * CPG):
                        st, sp = c == 0, c == NCHUNK - 1
                        nc.tensor.matmul(
                            pa[p_][:], esc[p_][:, c : c + 1], xt[p_][:, c, 0:512],
                            start=st, stop=sp,
                        )
                        mm = nc.tensor.matmul(
                            pb[p_][:], esc[p_][:, c : c + 1], xt[p_][:, c, 512:1024],
                            start=st, stop=sp,
                        )
                mm.then_inc(pe_pool, 1)

    return nc


_NC_CACHE: dict = {}


def _build(reps: int = 1, **kw) -> bass.Bass:
    key = (reps, tuple(sorted(kw.items())))
    if key not in _NC_CACHE:
        _NC_CACHE[key] = _build_nc(reps, **kw)
    return _NC_CACHE[key]


def _in_maps(x, w, b):
    x16 = np.asarray(x, dtype=np.float16)
    w16 = np.ascontiguousarray(np.asarray(w, dtype=np.float16))
    b = np.ascontiguousarray(np.asarray(b, dtype=np.float32))
    return [
        {
            "x": x16[c * BPC : (c + 1) * BPC],
            "w": w16,
            "bias": b,
            "bias_s": b / D,
        }
        for c in range(N_CORES)
    ]


def kernel(**inputs):
    x = np.asarray(inputs["inputs"], dtype=np.float32)
    w = np.asarray(inputs["att_weight"], dtype=np.float32)
    b = np.asarray(inputs["att_bias"], dtype=np.float32)
    nc = _build()
    res = run_bass_kernel_spmd(nc, _in_maps(x, w, b), list(range(N_CORES)))
    return np.concatenate([r["out"] for r in res.results], axis=0)

